# revision 15
# baseline (speedup 1.0000x reference)
"""Conv2D 3x3 (B=32, C=128, H=W=56 -> OC=256) via 1D Winograd F(2,3) on 8 cores.

Data-parallel over batch (4 images/core). Width dim uses Winograd F(2,3):
output col pair (2t, 2t+1) needs 4 transformed inputs x~[j] = B^T d over
padded cols 2t..2t+3; height dim stays direct (3 kh shift-taps accumulated
in PSUM). Per 14-row output block and oc-tile: 12 matmuls (4 j-planes x 3
kh) of free dim 392 (14 rows x 28 tile-cols) -> 2/3 the PE columns of the
direct 9-tap conv (62us vs 92us PE floor per core). Measured ~95us total
vs the 118.8us direct-conv baseline.

Host side does the LINEAR data prep: zero-pad, the width input transform
(4 shifted +/- passes; doubles input DMA bytes but the on-chip vector
engines can't spare the ~60us it costs there), the weight transform
w~[j] in {w0, (w0+w1+w2)/2, (w0-w1+w2)/2, w2} per kh laid out [C, 12, OC]
f16 in matmul need-order, and the final even/odd column interleave of the
separately-stored a/b output planes.

On-chip structure (all engines ~90% busy, measured rates in ns/392-elem
pass: ACT 586, DVE STT 565, GpSimd TT 1014; TensorTensor or f16-out on
DVE runs ~2x slower; GpSimd cannot touch PSUM):
  PE:     per group, 12 matmuls into 4 SINGLE-BANK psum pools (plane
          rewrites then only wait that plane's one drain reader -- a
          shared tile would gate on the slowest reader + ~600ns sem-post
          latency per hop). j-order (1,2,3,0), kh-order (1,0,2): the
          start=True matmul always covers full rows (edge trims ride
          start=False) and planes stop in drain order.
  ACT:    s1 = Id(z1 + bias), s2 = Id(z2)      (its queue stays DMA-free)
  DVE:    b' = -z3 + s1;  a' = z0 + s1;  a = a' + s2 (alternate groups)
  GpSimd: b = b' - s2;    a on the other groups (SBUF-only ops)
  rings:  input chunks on ACT's hw-DGE ring, output DMAs alternate
          Sync hw-DGE / GpSimd sw-DGE, issues deferred 2 groups so a
          dma_start never head-of-line blocks a queue on unfinished data.
Warm-up matmuls burn the PE until the first input lands (~9.8-10.5us:
ring spin-up bound); tail-burn matmuls keep the core clock (DVFS) up
through the final drains, whose passes otherwise run ~3x slower once the
PE idles. Weight DMAs are split per j-group so each tap set lands just
ahead of its matmuls (one 576KB DMA landed 1.7us late); the first-group
feed is otherwise ring-byte-bound (~1.24MB needed within ~2.6us of ring
spin-up > 2x180GB/s) leaving a single ~1.4us stall.
"""

import os

import numpy as np

import concourse.bacc as bacc
import concourse.mybir as mybir
import concourse.tile as tile
from concourse import bass_utils

B, C, H, W_SP = 32, 128, 56, 56
OC, KH, KW = 256, 3, 3
N_CORES = 8
B_PER = B // N_CORES            # 4 images per core
HP = H + 2                      # padded rows (58)
N_J = 4                         # winograd positions per tile
TW = W_SP // 2                  # 28 tiles across width
# block layout: 4x14 rows per image. Tried and rejected: (18,18,10,10)
# re-saturates DVE in 504-free groups; 7-row first blocks for image 0
# halve the early-ring stall but add ~1us of LDWEIGHTS-floor time (24
# small matmuls vs 12) -- a wash.
BLOCKS = ((0, 14), (14, 14), (28, 14), (42, 14))


def _blocks_of(img):
    return BLOCKS

MAX_ROWS = max(nr for _, nr in BLOCKS)          # 18
MAX_CHUNK = MAX_ROWS + KH - 1                   # 20 padded rows
NFREE = MAX_ROWS * TW                           # 504 (<=512: one bank)
OC_TILES = OC // 128            # 2
HWO = H * W_SP                  # 3136
XT_FREE = N_J * HP * TW         # per-channel x~ elements per image (6496)

# tap emission order: j in (1,2,3,0), kh in (1,0,2). kh=1 first => the
# start=True matmul covers full rows. j-order staggers the PSUM plane
# stops so every drain pass except a'/a starts BEFORE the group's last
# matmul: s1(z1)@mm3, s2(z2)@mm6, b'(z3)@mm9; only a'(z0)+a trail the
# group (~1.2us on DVE < the 2-buf PSUM rotation tolerance).
J_ORDER = (1, 2, 3, 0)
KH_ORDER = (1, 0, 2)
N_WARMUP = int(os.environ.get("CONV_WARMUP", "6"))
N_TAILBURN = int(os.environ.get("CONV_TAILBURN", "28"))

_NC_CACHE: dict[str, object] = {}


def _build_nc(mode: str = "f16"):
    in_dt = mybir.dt.float16
    f32 = mybir.dt.float32
    nc = bacc.Bacc(
        "TRN2",
        target_bir_lowering=False,
        debug=False,
        enable_asserts=False,
        num_devices=N_CORES,
    )
    # host-transformed input: [img, C, j, padded_row, tile_col]
    xt_d = nc.dram_tensor(
        "xt", [B_PER, C, XT_FREE], in_dt, kind="ExternalInput"
    ).ap()
    wt = nc.dram_tensor("wt", [C, 12 * OC], in_dt, kind="ExternalInput").ap()
    bias = nc.dram_tensor(
        "bias", [128, 2 * OC_TILES], f32, kind="ExternalInput"
    ).ap()
    out = nc.dram_tensor("out", [B_PER, OC, HWO], f32, kind="ExternalOutput").ap()

    with tile.TileContext(nc) as tc:
        with (
            tc.tile_pool(name="xt", bufs=8) as xtpool,
            tc.tile_pool(name="wpool", bufs=1) as wpool,
            tc.tile_pool(name="bpool", bufs=1) as bpool,
            tc.tile_pool(name="s1p", bufs=8) as s1pool,
            tc.tile_pool(name="ap", bufs=4) as apool,
            tc.tile_pool(name="bp", bufs=4) as bppool,
            tc.tile_pool(name="opool", bufs=6) as opool,
            tc.tile_pool(name="ps0", bufs=2, space="PSUM") as pspool0,
            tc.tile_pool(name="ps1", bufs=2, space="PSUM") as pspool1,
            tc.tile_pool(name="ps2", bufs=2, space="PSUM") as pspool2,
            tc.tile_pool(name="ps3", bufs=2, space="PSUM") as pspool3,
        ):
            pspools = [pspool0, pspool1, pspool2, pspool3]
            # HAM warm-up: burn the PE while the lead-in DMAs land so the
            # real stream starts at the warm clock.
            wu = wpool.tile([C, 512], in_dt, tag="wu")
            nc.gpsimd.memset(wu[:], 0.0)
            psw = pspool0.tile([128, 512], f32, tag="z0")
            for i in range(N_WARMUP):
                nc.tensor.matmul(
                    psw[:, :],
                    wu[:, :128],
                    wu[:, :512],
                    start=(i == 0),
                    stop=(i == N_WARMUP - 1),
                )

            # lead-in DMAs: first input chunk (longest dep chain), the 3
            # j=1 weight taps, second chunk, remaining taps. Bias rides
            # GpSimd's software DGE.
            wsb = wpool.tile([C, 12, OC], in_dt, tag="wsb")
            wtv = wt.rearrange("c (k m) -> c k m", m=OC)
            bsb = bpool.tile([128, 2 * OC_TILES], f32, tag="bsb")

            chunks = [
                (img, blk)
                for img in range(B_PER)
                for blk in range(len(_blocks_of(img)))
            ]

            def chunk_dma(ci, split=False, eng=None):
                img, blk = chunks[ci]
                xv = xt_d[img].rearrange("c (j h w) -> c j h w", j=N_J, w=TW)
                xc = xtpool.tile([C, N_J, MAX_CHUNK, TW], in_dt, tag="xc")
                r0, nr = _blocks_of(img)[blk]
                cr = nr + KH - 1
                # input chunks ride the ACT ring (its queue is just s1/s2)
                # so they never queue behind output transfers on Sync
                eng = eng or nc.scalar
                if split:
                    # lead-in: land planes in matmul need order (J_ORDER)
                    for j in J_ORDER:
                        eng.dma_start(
                            xc[:, j, :cr, :], xv[:, j, r0 : r0 + cr, :]
                        )
                else:
                    eng.dma_start(
                        xc[:, :, :cr, :], xv[:, :, r0 : r0 + cr, :]
                    )
                return xc

            xc_bufs = {}
            xc_bufs[0] = chunk_dma(0, split=True)
            nc.sync.dma_start(wsb[:, 0:3, :], wtv[:, 0:3, :])
            # per-j-group weight DMAs: a single w[3:12] lands ~14.3us, 1.7us
            # after the j3-tap matmuls need it; split, each lands in time
            nc.sync.dma_start(wsb[:, 3:6, :], wtv[:, 3:6, :])
            nc.sync.dma_start(wsb[:, 6:9, :], wtv[:, 6:9, :])
            nc.sync.dma_start(wsb[:, 9:12, :], wtv[:, 9:12, :])
            xc_bufs[1] = chunk_dma(1)
            xc_bufs[2] = chunk_dma(2)
            xc_bufs[3] = chunk_dma(3)
            nc.gpsimd.dma_start(bsb[:], bias[:])


            add = mybir.AluOpType.add
            mult = mybir.AluOpType.mult

            gi = 0  # group counter for DMA-ring alternation
            pending_dmas = []  # (dst, src, engine) deferred 2 groups so the
            # dma_start never head-of-line blocks s1/s1d (ACT) or chunk
            # prefetch (Sync) behind an unfinished ot tile
            for ci, (img, blk) in enumerate(chunks):
                if ci + 4 < len(chunks):
                    xc_bufs[ci + 4] = chunk_dma(ci + 4)
                xc = xc_bufs.pop(ci)

                is_last_chunk = ci == len(chunks) - 1
                blk0, nrows = _blocks_of(img)[blk]
                nfree = nrows * TW
                for oc_t in range(OC_TILES):
                    # per-plane PSUM pools: a rewrite of plane j only waits
                    # plane j's reader (tile-level WAR tracking would gate
                    # the whole group on the last drain pass + ~600ns
                    # sem-post latency per cross-engine hop)
                    if True:
                        psj = {}
                        for j in J_ORDER:
                            ztile = pspools[j].tile([128, 512], f32, tag=f"z{j}")
                            psj[j] = ztile
                        for jj, j in enumerate(J_ORDER):
                            for ki, kh in enumerate(KH_ORDER):
                                r0 = 1 if (blk0 == 0 and kh == 0) else 0
                                r1 = (
                                    nrows - 1
                                    if (blk0 + nrows == H and kh == 2)
                                    else nrows
                                )
                                rhs = xc[:, j, kh + r0 : kh + r1, :]
                                lhsT = wsb[
                                    :, jj * 3 + ki, oc_t * 128 : (oc_t + 1) * 128
                                ]
                                nc.tensor.matmul(
                                    psj[j][:, r0 * TW : r1 * TW],
                                    lhsT,
                                    rhs,
                                    start=(ki == 0),
                                    stop=(ki == len(KH_ORDER) - 1),
                                )

                        z = [psj[j][:, :nfree] for j in range(4)]
                        # Shallow 2-hop drain graph (deep cross-engine chains
                        # cascade into a latency-bound pipeline): ACT drains
                        # z1 and z2 independently; DVE's three STT passes and
                        # GpSimd's one SBUF-only TT hang off s1/s2.
                        s1 = s1pool.tile([128, NFREE], f32, tag="s1")
                        nc.scalar.activation(
                            s1[:, :nfree],
                            z[1],
                            mybir.ActivationFunctionType.Identity,
                            bias=bsb[:, oc_t : oc_t + 1],
                        )
                        s2 = s1pool.tile([128, NFREE], f32, tag="s2")
                        nc.scalar.activation(
                            s2[:, :nfree],
                            z[2],
                            mybir.ActivationFunctionType.Identity,
                        )
                        ap_ = apool.tile([128, NFREE], f32, tag="ap")
                        bp_ = bppool.tile([128, NFREE], f32, tag="bp")
                        # out keeps a/b planes separate (host interleaves the
                        # even/odd cols) so every vector pass is contiguous;
                        # fp32 out (TENSOR_TENSOR and f16-out passes run at
                        # half rate, ~1.05us vs 0.57us for fp32 STT).
                        ot = opool.tile([128, 2, NFREE], f32, tag="ot")
                        # a = z0+s1+z2 -> plane 0; b = s1-z2-z3 -> plane 1
                        nc.vector.scalar_tensor_tensor(
                            bp_[:, :nfree], z[3], -1.0, s1[:, :nfree], mult, add
                        )
                        nc.gpsimd.tensor_sub(
                            ot[:, 1, :nfree], bp_[:, :nfree], s2[:, :nfree]
                        )
                        nc.vector.scalar_tensor_tensor(
                            ap_[:, :nfree], z[0], 0.0, s1[:, :nfree], add, add
                        )
                        # DVE's 3rd STT pass saturates it (102%); the final
                        # SBUF-only combine alternates onto GpSimd
                        if gi % 2:
                            nc.gpsimd.tensor_add(
                                ot[:, 0, :nfree], ap_[:, :nfree], s2[:, :nfree]
                            )
                        else:
                            nc.vector.scalar_tensor_tensor(
                                ot[:, 0, :nfree], ap_[:, :nfree], 0.0,
                                s2[:, :nfree], add, add,
                            )

                        ov = out[img].rearrange(
                            "o (e h w) -> o e h w", e=2, w=TW
                        )
                        if is_last_chunk:
                            dma_eng = nc.scalar if gi % 2 else nc.sync
                        else:
                            dma_eng = nc.gpsimd if gi % 2 else nc.sync
                        pending_dmas.append(
                            (
                                ov[
                                    oc_t * 128 : (oc_t + 1) * 128,
                                    :,
                                    blk0 : blk0 + nrows,
                                    :,
                                ],
                                ot[:, :, :nfree],
                                dma_eng,
                            )
                        )
                        keep = 0 if is_last_chunk else 2
                        while len(pending_dmas) > keep:
                            dst, src, eng = pending_dmas.pop(0)
                            eng.dma_start(dst, src)
                        gi += 1
            # tail burn: dummy matmuls keep the core clock (DVFS) up while
            # the last groups drain + their DMAs fly -- with the PE idle the
            # vector drains run ~3x slower (2.4us stt vs 0.57us warm)
            psb = pspool1.tile([128, 512], f32, tag="z1")
            for i in range(N_TAILBURN):
                nc.tensor.matmul(
                    psb[:, :],
                    wu[:, :128],
                    wu[:, :512],
                    start=(i == 0),
                    stop=(i == N_TAILBURN - 1),
                )
            # final flush rides the HW rings (ACT ring is free of chunk
            # DMAs by now; GpSimd swdge transfers are ~2x slower)
            for fi, (dst, src, eng) in enumerate(pending_dmas):
                (nc.scalar if fi % 2 else nc.sync).dma_start(dst, src)
    nc.compile()
    return nc


def _get_nc(mode: str):
    nc = _NC_CACHE.get(mode)
    if nc is None:
        nc = _build_nc(mode)
        _NC_CACHE[mode] = nc
    return nc


def kernel(x: np.ndarray, W: np.ndarray, b: np.ndarray) -> np.ndarray:
    x = np.asarray(x, dtype=np.float32)
    W = np.asarray(W, dtype=np.float32)
    b = np.asarray(b, dtype=np.float32)
    in_np_dt = np.float16

    # zero-pad x spatially, then host-side 1D Winograd input transform
    # along width: d_k = padded col 2t+k; x~ planes j0..j3.
    xp = np.zeros((B, C, HP, H + 2), dtype=np.float32)
    xp[:, :, 1:-1, 1:-1] = x
    d0 = xp[:, :, :, 0:56:2]
    d1 = xp[:, :, :, 1:57:2]
    d2 = xp[:, :, :, 2:58:2]
    d3 = xp[:, :, :, 3:58:2]
    xt = np.empty((B, C, N_J, HP, TW), dtype=in_np_dt)
    xt[:, :, 0] = d0 - d2
    xt[:, :, 1] = d1 + d2
    xt[:, :, 2] = d2 - d1
    xt[:, :, 3] = d1 - d3
    xt = xt.reshape(N_CORES, B_PER, C, XT_FREE)

    # Winograd-transform W along kw per kh: wt[c, pos, oc] with pos in
    # matmul need-order (j in 1,0,2,3) x (kh in 1,0,2)
    wf = W.reshape(OC, C, KH, KW)
    w0, w1, w2 = wf[:, :, :, 0], wf[:, :, :, 1], wf[:, :, :, 2]
    wj = {
        0: w0,
        1: (w0 + w1 + w2) * 0.5,
        2: (w0 - w1 + w2) * 0.5,
        3: w2,
    }  # each [OC, C, KH]
    taps = []
    for j in J_ORDER:
        for kh in KH_ORDER:
            taps.append(wj[j][:, :, kh].T)  # [C, OC]
    wt = np.ascontiguousarray(
        np.stack(taps, axis=1).reshape(C, 12 * OC)
    ).astype(in_np_dt)
    b_cols = b.reshape(OC_TILES, 128).T
    bias = np.ascontiguousarray(
        np.concatenate([b_cols, 2.0 * b_cols], axis=1)
    ).astype(np.float32)

    nc = _get_nc("f16")
    in_maps = [
        {"xt": np.ascontiguousarray(xt[i]), "wt": wt, "bias": bias}
        for i in range(N_CORES)
    ]
    trace = os.environ.get("CONV_TRACE", "") not in ("", "0")
    try:
        res = bass_utils.run_bass_kernel_spmd(
            nc, in_maps, core_ids=list(range(N_CORES)), trace=trace
        )
    except Exception:
        import time

        time.sleep(2.0)
        res = bass_utils.run_bass_kernel_spmd(
            nc, in_maps, core_ids=list(range(N_CORES)), trace=trace
        )
    kernel._last_results = res
    out = np.stack([res.results[i]["out"] for i in range(N_CORES)])
    # device emits [img, oc, {a,b}, h, tw]; interleave a/b into even/odd cols
    out = out.reshape(B, OC, 2, H, TW).astype(np.float32)
    full = np.empty((B, OC, H, W_SP), dtype=np.float32)
    full[:, :, :, 0::2] = out[:, :, 0]
    full[:, :, :, 1::2] = out[:, :, 1]
    return full
